# revision 27
# baseline (speedup 1.0000x reference)
"""Trainium2 Bass kernel for nn_Encoder_81595788689580.

Attention-gated GRU encoder: per time step
    w1 = h @ attn1_W.T + attn1_b
    w2 = x_t @ attn2_W.T + attn2_b
    v  = tanh(w1 + w2) @ attn3_W.T + attn3_b
    alpha = softmax(v, axis=feature)
    wx = x_t * alpha
    GRU cell (r, z, n) -> h_new
Output: [B, T, H] hidden states.

Strategy (8 NeuronCores, data-parallel over batch):
  - batch 4096 -> 512 rows per core; all weights replicated; everything
    stored TRANSPOSED on chip (features on partitions, batch on the free
    dim); feature dim I=320 zero-padded to 384 = 3x128 partition blocks.
  - fp8 e4m3 matmuls with DoubleRow perf mode: block pairs {0,1} contract
    256 rows per instruction at 2 fp8 MACs/PE/cycle; the 64-row tails run
    as regular fp8 matmuls (FWL fast weight loads). h is stored fp16 (it
    IS the output and fp8 storage alone costs ~3% rel err) with an fp8
    shadow copy used as the matmul moving operand. The W_hh n-rows matmul
    stays fp16: the n path goes through tanh at slope 1 and dominates the
    fp8 error budget (numpy-sim: full fp8 3.6e-2, this mix 5.0e-3).
  - biases ride the matmuls: x pad lane 320 is set to 1.0 host-side and
    weight row 320 carries the bias of each matmul (attn1_b+attn2_b for
    ps_u, attn3_b for ps_v, b_ih+b_hh for r/z, b_ih for i_n). w2t[320,320]
    = 20 makes u[320] = tanh(20) = 1; w3t[320,320] = 3 cancels the exp
    range shift so ev[320] = 1 and wx[320] = 1 (the rinv multiply skips
    that lane; the softmax-denominator ones matrix excludes it). This
    turns 11 per-block ACT ops per chunk-step into 5 whole-tensor ones.
  - softmax denominator is an all-ones stationary matmul broadcasting the
    per-column sum to all partitions; max-subtraction is skipped (exp is
    shifted -3 to fit e4m3's 240 max); sigmoid is 0.5*tanh(x/2)+0.5 so
    every ACT op stays in the exp_and_others table set.
  - the 512-row batch runs as 2 chunks of 256 so the two recurrences
    pipeline against each other across engines.
"""

import numpy as np

B, T, I, H = 4096, 24, 320, 256
NCORES = 8
BS = B // NCORES          # 512 rows per core
IP = 384                  # I padded to 3*128
KI = IP // 128            # 3 feature blocks
KH = H // 128             # 2 hidden blocks
G = 3 * H                 # 768 gate rows
G2 = 2 * H                # r,z gate rows
NCHUNK = 2
CB = BS // NCHUNK         # 256 batch columns per chunk
LB = I                    # bias lane (== 320), row 64 of block 2

_STATE = {}


def _np_f8():
    from concourse import mybir
    return mybir.dt.np(mybir.dt.float8e4)


def _build(t_steps=T):
    import concourse.bass as bass
    import concourse.tile as tile
    from concourse import bacc, mybir

    f32 = mybir.dt.float32
    f16 = mybir.dt.float16
    f8 = mybir.dt.float8e4
    AF = mybir.ActivationFunctionType
    OP = mybir.AluOpType
    DR = mybir.MatmulPerfMode.DoubleRow

    nc = bacc.Bacc("TRN2", target_bir_lowering=False, debug=False,
                   num_devices=NCORES)

    xT = nc.dram_tensor("xT", [t_steps, 128, KI, BS], f8,
                        kind="ExternalInput").ap()
    h0T16 = nc.dram_tensor("h0T16", [128, KH, BS], f16,
                           kind="ExternalInput").ap()
    h0T8 = nc.dram_tensor("h0T8", [128, KH, BS], f8,
                          kind="ExternalInput").ap()
    wat1 = nc.dram_tensor("wat1", [128, KH, IP], f8, kind="ExternalInput").ap()
    wat2 = nc.dram_tensor("wat2", [128, KI, IP], f8, kind="ExternalInput").ap()
    wat3 = nc.dram_tensor("wat3", [128, KI, IP], f8, kind="ExternalInput").ap()
    wih = nc.dram_tensor("wih", [128, KI, G], f8, kind="ExternalInput").ap()
    whh8 = nc.dram_tensor("whh8", [128, KH, G2], f8,
                          kind="ExternalInput").ap()
    whh16n = nc.dram_tensor("whh16n", [128, KH, H], f16,
                            kind="ExternalInput").ap()
    onesw = nc.dram_tensor("onesw", [128, KI, 128], f8,
                           kind="ExternalInput").ap()
    # b_hh n-rows pre-halved, [128, KH]
    bhn_d = nc.dram_tensor("bhn_half", [128, KH], f32,
                           kind="ExternalInput").ap()
    # b_ih n-rows exact (fp8 would bias n systematically every step)
    bin_d = nc.dram_tensor("bin_f32", [128, KH], f32,
                           kind="ExternalInput").ap()
    outT = nc.dram_tensor("outT", [t_steps, 128, KH, BS], f16,
                          kind="ExternalOutput").ap()

    with tile.TileContext(nc) as tc:
        with tc.tile_pool(name="const", bufs=1) as cp, \
             tc.tile_pool(name="xs", bufs=1) as xp, \
             tc.tile_pool(name="hs", bufs=1) as hp, \
             tc.tile_pool(name="wk", bufs=1) as wp, \
             tc.tile_pool(name="ps", bufs=1, space="PSUM") as pp:

            w1t = cp.tile([128, KH, IP], f8)
            w2t = cp.tile([128, KI, IP], f8)
            w3t = cp.tile([128, KI, IP], f8)
            wiht = cp.tile([128, KI, G], f8)
            whh8t = cp.tile([128, KH, G2], f8)
            whhnt = cp.tile([128, KH, H], f16)
            onest = cp.tile([128, KI, 128], f8)
            bhnt = cp.tile([128, KH], f32)
            bint = cp.tile([128, KH], f32)
            bv3 = cp.tile([128, 1], f32)
            nc.vector.memset(bv3[:], -3.0)
            # h0 + step-0 x first (they gate the first matmuls), then
            # weights ordered by first use, alternating the two HWDGE rings
            h16c, h8c = [], []
            for ci in range(NCHUNK):
                cs = slice(ci * CB, (ci + 1) * CB)
                h16 = hp.tile([128, KH, CB], f16, name=f"h16_{ci}",
                              tag=f"h16{ci}", bufs=2)
                nc.scalar.dma_start(out=h16[:], in_=h0T16[:, :, cs])
                h8 = hp.tile([128, KH, CB], f8, name=f"h8_{ci}",
                             tag=f"h8{ci}", bufs=2)
                nc.scalar.dma_start(out=h8[:], in_=h0T8[:, :, cs])
                h16c.append(h16)
                h8c.append(h8)
            x_pre = xp.tile([128, KI, BS], f8, name="x_pre", tag="x", bufs=4)
            nc.sync.dma_start(out=x_pre[:], in_=xT[0])
            for i, (dst, src) in enumerate([
                    (w2t, wat2), (w1t, wat1), (w3t, wat3), (onest, onesw),
                    (whhnt, whh16n), (whh8t, whh8), (wiht, wih),
                    (bhnt, bhn_d), (bint, bin_d)]):
                eng = nc.sync if i % 2 == 0 else nc.scalar
                eng.dma_start(out=dst[:], in_=src)

            def ms(m):
                return slice(m * 128, (m + 1) * 128)

            # stride-0 broadcast of the pre-halved b_hh n-bias to [128,KH,CB]
            _b = bhnt[:]
            bhn_bc = bass.AP(tensor=_b.tensor, offset=_b.offset,
                             ap=[_b.ap[0], _b.ap[1], [0, CB]])

            # ---- software-pipelined x-front: create step t's x tile +
            # ps_u tiles and issue the x-only matmuls for chunk A (all 3
            # blocks) and chunk B (blocks 0,1). Hoisted into the PREVIOUS
            # step between chunk A's and chunk B's gate phases, so the PE
            # has h-independent work while the previous DVE tail drains.
            # The psu tag ring (bufs=5) only ever waits on psu slots freed
            # by u ACTs a full step earlier, so the hoist cannot deadlock.
            def make_xfront(t):
                if t == 0:
                    x_t = x_pre
                else:
                    x_t = xp.tile([128, KI, BS], f8, name=f"x_{t}",
                                  tag="x", bufs=4)
                    nc.sync.dma_start(out=x_t[:], in_=xT[t])
                ps_us = [[pp.tile([128, CB], f32, name=f"psu{m}_{t}_{ci}",
                                  tag="aps", bufs=5) for m in range(KI)]
                         for ci in range(NCHUNK)]

                def psu_x(ci, m):
                    cs = slice(ci * CB, (ci + 1) * CB)
                    nc.tensor.matmul(ps_us[ci][m][:], w2t[:, 0:2, ms(m)],
                                     x_t[:, 0:2, cs], start=True,
                                     stop=False, perf_mode=DR)
                    nc.tensor.matmul(ps_us[ci][m][:], w2t[:, 2, ms(m)],
                                     x_t[:, 2, cs], start=False, stop=False)

                for m in range(KI):
                    psu_x(0, m)
                psu_x(1, 0)
                psu_x(1, 1)
                return x_t, ps_us, psu_x

            nxt = make_xfront(0)
            for t in range(t_steps):
                x_t, ps_us, psu_x = nxt
                st = [{} for _ in range(NCHUNK)]

                def psu_h(ci, m):
                    nc.tensor.matmul(ps_us[ci][m][:], w1t[:, 0:2, ms(m)],
                                     h8c[ci][:, 0:2, :], start=False,
                                     stop=True, perf_mode=DR)

                def u_act(ci):
                    u = wp.tile([128, KI, CB], f8, name=f"u_{t}_{ci}",
                                tag="u", bufs=3)
                    for m in range(KI):
                        nc.scalar.activation(u[:, m, :], ps_us[ci][m][:],
                                             AF.Tanh)
                    st[ci].update(u=u)

                for m in range(KI):
                    psu_h(0, m)
                u_act(0)
                psu_x(1, 2)
                for m in range(KI):
                    psu_h(1, m)
                u_act(1)

                # ---- phases 2+3 run chunk-deep: A's softmax+gates+tail
                # issue before B's, so the DVE queue ahead of A's h8 (which
                # gates the next step's h-DRs) is as short as possible.
                # psv/den live on their own 3-slot ring so the hoisted
                # x-front's psu acquisitions never chain behind them.
                def ph2(ci):
                    cs = slice(ci * CB, (ci + 1) * CB)
                    u = st[ci]["u"]
                    ps_v = [pp.tile([128, CB], f32, name=f"psv{m}_{t}_{ci}",
                                    tag="aps", bufs=5) for m in range(KI)]
                    ev = wp.tile([128, KI, CB], f8, name=f"ev_{t}_{ci}",
                                 tag="ev", bufs=3)
                    for m in range(KI):
                        nc.tensor.matmul(ps_v[m][:], w3t[:, 0:2, ms(m)],
                                         u[:, 0:2, :], start=True,
                                         stop=False, perf_mode=DR)
                        nc.tensor.matmul(ps_v[m][:], w3t[:, 2, ms(m)],
                                         u[:, 2, :], start=False, stop=True)
                        nc.scalar.activation(ev[:, m, :], ps_v[m][:], AF.Exp,
                                             bias=bv3[:])
                    ps_den = pp.tile([128, CB], f32, name=f"psden_{t}_{ci}",
                                     tag="aps", bufs=5)
                    nc.tensor.matmul(ps_den[:], onest[:, 0:2, :],
                                     ev[:, 0:2, :], start=True, stop=False,
                                     perf_mode=DR)
                    nc.tensor.matmul(ps_den[:], onest[:, 2, :], ev[:, 2, :],
                                     start=False, stop=True)
                    rinv = wp.tile([128, CB], f32, name=f"rinv_{t}_{ci}",
                                   tag="rinv", bufs=3)
                    nc.vector.reciprocal_approx_fast(rinv[:], ps_den[:])
                    wx = wp.tile([128, KI, CB], f8, name=f"wx_{t}_{ci}",
                                 tag="wx", bufs=3)
                    nc.vector.tensor_mul(wx[:], x_t[:, :, cs], ev[:])
                    _r = rinv[:]
                    rrep = bass.AP(tensor=_r.tensor, offset=_r.offset,
                                   ap=[_r.ap[0], [0, 2], _r.ap[1]])
                    nc.vector.tensor_mul(wx[:, 0:2, :], wx[:, 0:2, :], rrep)
                    # block 2: scale only the 64 real lanes; lane 320
                    # (row 64) stays 1.0 to feed the gate bias row.
                    # gpsimd: runs parallel to the rrep mul, off DVE's queue
                    nc.gpsimd.tensor_mul(wx[0:64, 2, :], wx[0:64, 2, :],
                                         rinv[0:64, :])
                    st[ci].update(wx=wx)

                    # ---- phase 3: gate matmuls + GRU tail ----
                    h16 = h16c[ci]
                    h8 = h8c[ci]
                    # n-path h matmul in fp16 (most error-sensitive)
                    ps_hn = pp.tile([128, KH, CB], f32, name=f"pshn_{t}_{ci}",
                                    tag="gps", bufs=3)
                    for m in range(KH):
                        for k in range(KH):
                            nc.tensor.matmul(
                                ps_hn[:, m, :], whhnt[:, k, ms(m)],
                                h16[:, k, :], start=(k == 0),
                                stop=(k == KH - 1))
                    # t1h only needs ps_hn: issue it before the gate matmuls
                    # so the post-gate DVE chain is as short as possible
                    t1h = wp.tile([128, KH, CB], f16, name=f"t1h_{t}_{ci}",
                                  tag="t1h", bufs=3)
                    nc.vector.scalar_tensor_tensor(
                        t1h[:], ps_hn[:], 0.5, bhn_bc, OP.mult, OP.add)
                    ps_r = pp.tile([128, 2, CB], f32, name=f"psr_{t}_{ci}",
                                   tag="gps", bufs=3)
                    ps_z = pp.tile([128, 2, CB], f32, name=f"psz_{t}_{ci}",
                                   tag="gps", bufs=3)
                    # h-only DRs first (r/z tiles sit in different PSUM
                    # banks, so one group of each may be open at once):
                    # they keep the in-order PE stream fed while wx lands
                    nc.tensor.matmul(ps_r[:, 0, :], whh8t[:, 0:2, ms(0)],
                                     h8[:, 0:2, :], start=True, stop=False,
                                     perf_mode=DR)
                    nc.tensor.matmul(ps_z[:, 0, :], whh8t[:, 0:2, ms(2)],
                                     h8[:, 0:2, :], start=True, stop=False,
                                     perf_mode=DR)
                    # i_n matmuls before the r/z wx parts: s2 = q1 + ps_in
                    # wants ps_in closed as early as possible
                    ps_in = pp.tile([128, KH, CB], f32, name=f"psin_{t}_{ci}",
                                    tag="gps", bufs=3)
                    for m in range(KH):
                        nc.tensor.matmul(ps_in[:, m, :], wiht[:, 0:2, ms(4 + m)],
                                         wx[:, 0:2, :], start=True,
                                         stop=False, perf_mode=DR)
                        nc.tensor.matmul(ps_in[:, m, :], wiht[:, 2, ms(4 + m)],
                                         wx[:, 2, :], start=False, stop=True)
                    for mm_t, base in ((ps_r, 0), (ps_z, 2)):
                        nc.tensor.matmul(
                            mm_t[:, 0, :], wiht[:, 0:2, ms(base)],
                            wx[:, 0:2, :], start=False, stop=False,
                            perf_mode=DR)
                        nc.tensor.matmul(
                            mm_t[:, 0, :], wiht[:, 2, ms(base)],
                            wx[:, 2, :], start=False, stop=True)
                        nc.tensor.matmul(
                            mm_t[:, 1, :], whh8t[:, 0:2, ms(base + 1)],
                            h8[:, 0:2, :], start=True, stop=False,
                            perf_mode=DR)
                        nc.tensor.matmul(
                            mm_t[:, 1, :], wiht[:, 0:2, ms(base + 1)],
                            wx[:, 0:2, :], start=False, stop=False,
                            perf_mode=DR)
                        nc.tensor.matmul(
                            mm_t[:, 1, :], wiht[:, 2, ms(base + 1)],
                            wx[:, 2, :], start=False, stop=True)

                    g = wp.tile([128, 4, CB], f16, name=f"g_{t}_{ci}",
                                tag="g", bufs=3)
                    nc.scalar.activation(g[:, 0:2, :], ps_r[:], AF.Tanh,
                                         scale=0.5)
                    nc.scalar.activation(g[:, 2:4, :], ps_z[:], AF.Tanh,
                                         scale=0.5)
                    p_ = wp.tile([128, KH, CB], f16, name=f"p_{t}_{ci}",
                                 tag="p", bufs=3)
                    nc.vector.tensor_add(p_[:], ps_in[:], t1h[:])
                    t0h = wp.tile([128, KH, CB], f16, name=f"t0h_{t}_{ci}",
                                  tag="t0h", bufs=3)
                    nc.vector.tensor_mul(t0h[:], t1h[:], g[:, 0:2, :])
                    s2 = wp.tile([128, KH, CB], f16, name=f"s2_{t}_{ci}",
                                 tag="s2", bufs=3)
                    nc.vector.tensor_add(s2[:], t0h[:], p_[:])
                    n = wp.tile([128, KH, CB], f16, name=f"n_{t}_{ci}",
                                tag="n", bufs=3)
                    for m in range(KH):
                        nc.scalar.activation(n[:, m, :], s2[:, m, :], AF.Tanh,
                                             bias=bint[:, m:m + 1])

                    # zz/d on gpsimd: they run while DVE drains the n-path,
                    # and only tensor_scalar/tensor_sub/add/mul lower there
                    zz = wp.tile([128, KH, CB], f16, name=f"zz_{t}_{ci}",
                                 tag="zz", bufs=3)
                    nc.gpsimd.tensor_scalar(
                        out=zz[:], in0=g[:, 2:4, :], scalar1=0.5, scalar2=0.5,
                        op0=OP.mult, op1=OP.add)
                    d_ = wp.tile([128, KH, CB], f16, name=f"d_{t}_{ci}",
                                 tag="d", bufs=3)
                    nc.gpsimd.tensor_sub(d_[:], h16[:], n[:])
                    zd = wp.tile([128, KH, CB], f16, name=f"zd_{t}_{ci}",
                                 tag="zd", bufs=3)
                    nc.vector.tensor_mul(zd[:], zz[:], d_[:])
                    # h8 first on DVE: it gates the next step's h-DRs;
                    # h16 has slack (read in the next gates phase) -> gpsimd
                    h8n = hp.tile([128, KH, CB], f8, name=f"h8n_{t}_{ci}",
                                  tag=f"h8{ci}", bufs=2)
                    nc.vector.tensor_add(h8n[:], n[:], zd[:])
                    h16n = hp.tile([128, KH, CB], f16, name=f"h16n_{t}_{ci}",
                                   tag=f"h16{ci}", bufs=2)
                    nc.gpsimd.tensor_add(h16n[:], n[:], zd[:])
                    h16c[ci] = h16n
                    h8c[ci] = h8n

                    nc.sync.dma_start(out=outT[t][:, :, cs], in_=h16n[:])

                ph2(0)
                ph3(0)
                ph2(1)
                # hoist the next step's x-front here: its psu ring slots
                # are freed by already-issued work (u/ev ACTs of this
                # step), and PE chews ~3us of h-independent matmuls while
                # chunk A's DVE tail (which produces h8) drains
                if t + 1 < t_steps:
                    nxt = make_xfront(t + 1)
                ph3(1)

    nc.compile()
    return nc


# ---------------- host-side data prep ----------------

def _prep_core_inputs(x, h0, attn1_W, attn1_b, attn2_W, attn2_b, attn3_W,
                      attn3_b, W_ih, b_ih, W_hh, b_hh, t_steps=T):
    f4 = np.float32
    f8 = _np_f8()
    x = np.asarray(x, f4)
    h0 = np.asarray(h0, f4)

    A1 = np.asarray(attn1_W, f4)                       # [I, H]
    w1 = np.zeros((H, IP), f4)
    w1[:, :I] = A1.T                                   # lhsT[hh, ii]
    wat1 = np.ascontiguousarray(
        w1.reshape(KH, 128, IP).transpose(1, 0, 2)).astype(f8)

    A2 = np.asarray(attn2_W, f4)                       # [I, I] (out, in)
    w2 = np.zeros((IP, IP), f4)
    w2[:I, :I] = A2.T                                  # lhsT[in, out]
    w2[LB, :I] = np.asarray(attn1_b, f4) + np.asarray(attn2_b, f4)
    w2[LB, LB] = 20.0                                  # u[320] = tanh(20) = 1
    wat2 = np.ascontiguousarray(
        w2.reshape(KI, 128, IP).transpose(1, 0, 2)).astype(f8)

    A3 = np.asarray(attn3_W, f4)
    w3 = np.zeros((IP, IP), f4)
    w3[:I, :I] = A3.T
    w3[LB, :I] = np.asarray(attn3_b, f4)
    w3[LB, LB] = 3.0                # cancels the exp -3 shift: ev[320] = 1
    w3[LB, LB + 1:] = -192.0        # pad lanes: exp(-195) = 0
    wat3 = np.ascontiguousarray(
        w3.reshape(KI, 128, IP).transpose(1, 0, 2)).astype(f8)

    Wi = np.asarray(W_ih, f4)                          # [G, I]
    bi = np.asarray(b_ih, f4)
    bh = np.asarray(b_hh, f4)
    wi = np.zeros((IP, G), f4)
    wi[:I, :] = Wi.T
    wi[LB, :G2] = bi[:G2] + bh[:G2]                    # r/z biases
    # i_n bias NOT here: fp8-quantizing it biases n systematically every
    # step and the error integrates over T; it rides the n ACT instead
    wih = np.ascontiguousarray(
        wi.reshape(KI, 128, G).transpose(1, 0, 2)).astype(f8)

    Wh = np.asarray(W_hh, f4)                          # [G, H]
    whT = Wh.T                                         # [H, G]
    whh8 = np.ascontiguousarray(
        whT[:, :G2].reshape(KH, 128, G2).transpose(1, 0, 2)).astype(f8)
    whh16n = np.ascontiguousarray(
        whT[:, G2:].reshape(KH, 128, H).transpose(1, 0, 2)).astype(np.float16)

    ones = np.ones((128, KI, 128), f4)
    ones[64:, 2, :] = 0.0           # exclude lane 320 + pads from the denom
    onesw = ones.astype(f8)

    bhn_half = np.ascontiguousarray((bh[G2:] * 0.5).reshape(KH, 128).T)
    bin_f32 = np.ascontiguousarray(bi[G2:].reshape(KH, 128).T.astype(f4))

    x8 = x[:, :t_steps, :].astype(f8)
    xpad = np.zeros((B, t_steps, IP), f8)
    xpad[:, :, :I] = x8
    xpad[:, :, LB] = np.asarray(1.0, f8)               # bias lane
    # [NC, BS, T, KI, 128] -> [NC, T, 128, KI, BS]
    xr = xpad.reshape(NCORES, BS, t_steps, KI, 128).transpose(0, 2, 4, 3, 1)
    h0r16 = h0.astype(np.float16).reshape(
        NCORES, BS, KH, 128).transpose(0, 3, 2, 1)
    h0r8 = h0.astype(f8).reshape(NCORES, BS, KH, 128).transpose(0, 3, 2, 1)

    shared = dict(wat1=wat1, wat2=wat2, wat3=wat3, wih=wih, whh8=whh8,
                  whh16n=whh16n, onesw=onesw, bhn_half=bhn_half,
                  bin_f32=bin_f32)
    in_maps = []
    for c in range(NCORES):
        m = dict(shared)
        m["xT"] = np.ascontiguousarray(xr[c])
        m["h0T16"] = np.ascontiguousarray(h0r16[c])
        m["h0T8"] = np.ascontiguousarray(h0r8[c])
        in_maps.append(m)
    return in_maps


def _gather(results, t_steps=T):
    outs = []
    for c in range(NCORES):
        o = np.asarray(results[c]["outT"], np.float32)
        outs.append(o.transpose(3, 0, 2, 1).reshape(BS, t_steps, H))
    return np.ascontiguousarray(np.concatenate(outs, axis=0))


def _get_nc(t_steps=T):
    key = ("nc", t_steps)
    if key not in _STATE:
        _STATE[key] = _build(t_steps)
    return _STATE[key]


def run(inputs, trace=False, t_steps=T):
    from concourse.bass_utils import run_bass_kernel_spmd
    nc = _get_nc(t_steps)
    in_maps = _prep_core_inputs(t_steps=t_steps, **inputs)
    res = run_bass_kernel_spmd(nc, in_maps, list(range(NCORES)), trace=trace)
    return _gather(res.results, t_steps), res


def kernel(**inputs):
    out, _ = run(inputs, trace=False)
    return out


# revision 29
# speedup vs baseline: 1.0144x; 1.0144x over previous
"""Trainium2 Bass kernel for nn_Encoder_81595788689580.

Attention-gated GRU encoder: per time step
    w1 = h @ attn1_W.T + attn1_b
    w2 = x_t @ attn2_W.T + attn2_b
    v  = tanh(w1 + w2) @ attn3_W.T + attn3_b
    alpha = softmax(v, axis=feature)
    wx = x_t * alpha
    GRU cell (r, z, n) -> h_new
Output: [B, T, H] hidden states.

Strategy (8 NeuronCores, data-parallel over batch):
  - batch 4096 -> 512 rows per core; all weights replicated.
  - everything stored TRANSPOSED on chip: features on partitions, batch on
    the free dim. Every matmul is weights-stationary with batch as the
    moving dim, biases become per-partition ACT bias vectors, and no
    transposes are ever needed on device (host pre-/post-transposes).
  - feature dim I=320 zero-padded to 384 = 3x128 partition blocks; padded
    attn3_b rows are -1e4 so exp() of pad rows is exactly 0 and the
    softmax denominator is unaffected.
  - softmax over features is a partition reduction: an all-ones stationary
    matmul broadcasts the per-column denominator into all 128 partitions
    of one PSUM tile; max-subtraction is skipped (|v| <= ~8 in practice,
    exp stays finite, softmax is shift-invariant).
  - sigmoid is computed as 0.5*tanh(x/2)+0.5 so every ACT op uses the
    exp_and_others table set -- avoids ~2.7us ACT table swaps per step.
  - matmuls in fp16 (1 PE cycle/row, fast weight loads) with fp32 PSUM
    accumulation; attn3_b carries a -2 shift for fp16 exp range. DT="f32r" switches
    to float32r matmuls (~10x lower error, ~1.7x slower weight loads).
  - the 512-row batch runs as 2 independent chunks of 256 so the two
    recurrences pipeline against each other across engines.
"""

import numpy as np

B, T, I, H = 4096, 24, 320, 256
NCORES = 8
BS = B // NCORES          # 512 rows per core
IP = 384                  # I padded to 3*128
KI = IP // 128            # 3 feature blocks
KH = H // 128             # 2 hidden blocks
G = 3 * H                 # 768 gate rows
NCHUNK = 2
CB = BS // NCHUNK         # 256 batch columns per chunk

DT = "f16"                # "f16" | "f32r"

_STATE = {}


def _np_dt(mdt):
    from concourse import mybir
    return mybir.dt.np(mdt)


def _dts():
    from concourse import mybir
    if DT == "f16":
        return mybir.dt.float16, mybir.dt.float16
    return mybir.dt.float32r, mybir.dt.float32r


def _build(t_steps=T):
    import concourse.bass as bass
    import concourse.tile as tile
    from concourse import bacc, mybir

    f32 = mybir.dt.float32
    F8D = mybir.dt.float8e4
    DRM = mybir.MatmulPerfMode.DoubleRow
    MMD, EVD = _dts()
    AF = mybir.ActivationFunctionType
    OP = mybir.AluOpType

    nc = bacc.Bacc("TRN2", target_bir_lowering=False, debug=False,
                   num_devices=NCORES)

    xT = nc.dram_tensor("xT", [t_steps, 128, KI, BS], MMD,
                        kind="ExternalInput").ap()
    h0T = nc.dram_tensor("h0T", [128, KH, BS], MMD, kind="ExternalInput").ap()
    h0T8 = nc.dram_tensor("h0T8", [128, KH, BS], F8D, kind="ExternalInput").ap()
    wat1 = nc.dram_tensor("wat1", [128, KH, IP], F8D, kind="ExternalInput").ap()
    wat2 = nc.dram_tensor("wat2", [128, KI, IP], MMD, kind="ExternalInput").ap()
    wat3 = nc.dram_tensor("wat3", [128, KI, IP], MMD, kind="ExternalInput").ap()
    wih = nc.dram_tensor("wih", [128, KI, G], MMD, kind="ExternalInput").ap()
    whh8 = nc.dram_tensor("whh8", [128, KH, 2 * H], F8D,
                          kind="ExternalInput").ap()
    whhn = nc.dram_tensor("whhn", [128, KH, H], MMD,
                          kind="ExternalInput").ap()
    onesw = nc.dram_tensor("onesw", [128, 128], EVD, kind="ExternalInput").ap()
    bias_u_d = nc.dram_tensor("bias_u", [128, KI], f32, kind="ExternalInput").ap()
    bias_v_d = nc.dram_tensor("bias_v", [128, KI], f32, kind="ExternalInput").ap()
    # rz bias pre-halved for the tanh-based sigmoid
    bias_rzh_d = nc.dram_tensor("bias_rzh", [128, 4], f32,
                                kind="ExternalInput").ap()
    bias_hn_d = nc.dram_tensor("bias_hn", [128, 2], f32, kind="ExternalInput").ap()
    bias_in_d = nc.dram_tensor("bias_in", [128, 2], f32, kind="ExternalInput").ap()
    outT = nc.dram_tensor("outT", [t_steps, 128, KH, BS], MMD,
                          kind="ExternalOutput").ap()

    def fv(ap):
        # readable view for DVE of matmul-dtype tiles
        if DT == "f32r":
            return ap.bitcast(f32)
        return ap

    with tile.TileContext(nc) as tc:
        with tc.tile_pool(name="const", bufs=1) as cp, \
             tc.tile_pool(name="xs", bufs=1) as xp, \
             tc.tile_pool(name="hs", bufs=1) as hp, \
             tc.tile_pool(name="wk", bufs=1) as wp, \
             tc.tile_pool(name="ps", bufs=1, space="PSUM") as pp:

            w1t = cp.tile([128, KH, IP], F8D)
            w2t = cp.tile([128, KI, IP], MMD)
            w3t = cp.tile([128, KI, IP], MMD)
            wiht = cp.tile([128, KI, G], MMD)
            whh8t = cp.tile([128, KH, 2 * H], F8D)
            whhnt = cp.tile([128, KH, H], MMD)
            onest = cp.tile([128, 128], EVD)
            bu = cp.tile([128, KI], f32)
            bv = cp.tile([128, KI], f32)
            brzh = cp.tile([128, 4], f32)
            bhn = cp.tile([128, 2], f32)
            bin_ = cp.tile([128, 2], f32)
            # h0 + step-0 x first (they gate the first matmuls), then
            # weights ordered by first use, alternating the two HWDGE rings
            hcur, h8cur = [], []
            for ci in range(NCHUNK):
                hc = hp.tile([128, KH, CB], MMD, name=f"h_{ci}",
                             tag=f"h{ci}", bufs=2)
                nc.scalar.dma_start(
                    out=hc[:], in_=h0T[:, :, ci * CB:(ci + 1) * CB])
                hcur.append(hc)
                h8c = hp.tile([128, KH, CB], F8D, name=f"h8_{ci}",
                              tag=f"h8{ci}", bufs=2)
                nc.scalar.dma_start(
                    out=h8c[:], in_=h0T8[:, :, ci * CB:(ci + 1) * CB])
                h8cur.append(h8c)
            x_pre = xp.tile([128, KI, BS], MMD, name="x_pre", tag="x", bufs=4)
            nc.sync.dma_start(out=x_pre[:], in_=xT[0])
            for i, (dst, src) in enumerate([
                    (w2t, wat2), (w1t, wat1), (bu, bias_u_d),
                    (w3t, wat3), (bv, bias_v_d), (onest, onesw),
                    (whh8t, whh8), (whhnt, whhn), (wiht, wih),
                    (brzh, bias_rzh_d), (bhn, bias_hn_d),
                    (bin_, bias_in_d)]):
                eng = nc.sync if i % 2 == 0 else nc.scalar
                eng.dma_start(out=dst[:], in_=src)

            def ms(m):
                return slice(m * 128, (m + 1) * 128)

            for t in range(t_steps):
                if t == 0:
                    x_t = x_pre
                else:
                    x_t = xp.tile([128, KI, BS], MMD, name=f"x_{t}",
                                  tag="x", bufs=4)
                    nc.sync.dma_start(out=x_t[:], in_=xT[t])

                st = [{} for _ in range(NCHUNK)]

                # ---- phase 1: h-gate matmuls + attention stage 1 ----
                for ci in range(NCHUNK):
                    cs = slice(ci * CB, (ci + 1) * CB)
                    h = hcur[ci]
                    ps_u = [pp.tile([128, CB], f32,
                                    name=f"psu{m}_{t}_{ci}", tag="aps",
                                    bufs=5) for m in range(KI)]
                    h8 = h8cur[ci]
                    for m in range(KI):
                        for k in range(KI):
                            nc.tensor.matmul(
                                ps_u[m][:], w2t[:, k, ms(m)],
                                x_t[:, k, cs], start=(k == 0), stop=False)
                        nc.tensor.matmul(
                            ps_u[m][:], w1t[:, 0:2, ms(m)],
                            h8[:, 0:2, :], start=False, stop=True,
                            perf_mode=DRM)
                    u = wp.tile([128, KI, CB], MMD, name=f"u_{t}_{ci}",
                                tag="u", bufs=3)
                    for m in range(KI):
                        nc.scalar.activation(u[:, m, :], ps_u[m][:],
                                             AF.Tanh, bias=bu[:, m:m + 1])
                    st[ci].update(u=u)

                # ---- phase 2: v, softmax, wx ----
                for ci in range(NCHUNK):
                    cs = slice(ci * CB, (ci + 1) * CB)
                    u = st[ci]["u"]
                    ps_v = [pp.tile([128, CB], f32,
                                    name=f"psv{m}_{t}_{ci}", tag="aps",
                                    bufs=5) for m in range(KI)]
                    for m in range(KI):
                        for k in range(KI):
                            nc.tensor.matmul(
                                ps_v[m][:], w3t[:, k, ms(m)],
                                u[:, k, :], start=(k == 0), stop=(k == KI - 1))
                    ev = wp.tile([128, KI, CB], EVD, name=f"ev_{t}_{ci}",
                                 tag="ev", bufs=3)
                    for m in range(KI):
                        nc.scalar.activation(ev[:, m, :], ps_v[m][:],
                                             AF.Exp, bias=bv[:, m:m + 1])
                    ps_den = pp.tile([128, CB], f32, name=f"psden_{t}_{ci}",
                                     tag="aps", bufs=5)
                    for k in range(KI):
                        nc.tensor.matmul(ps_den[:], onest[:], ev[:, k, :],
                                         start=(k == 0), stop=(k == KI - 1))
                    rinv = wp.tile([128, CB], f32, name=f"rinv_{t}_{ci}",
                                   tag="rinv", bufs=3)
                    nc.vector.reciprocal_approx_fast(rinv[:], ps_den[:])
                    rinv16 = wp.tile([128, CB], MMD, name=f"rinv16_{t}_{ci}",
                                     tag="rinv16", bufs=3)
                    nc.vector.tensor_copy(rinv16[:], rinv[:])
                    wx = wp.tile([128, KI, CB], MMD, name=f"wx_{t}_{ci}",
                                 tag="wx", bufs=3)
                    nc.vector.tensor_mul(wx[:], fv(x_t[:, :, cs]), fv(ev[:]))
                    _r = rinv16[:]
                    nc.vector.tensor_mul(wx[:, 0, :], fv(wx[:, 0, :]), _r)
                    rrep = bass.AP(tensor=_r.tensor, offset=_r.offset,
                                   ap=[_r.ap[0], [0, KI - 1], _r.ap[1]])
                    nc.vector.tensor_mul(wx[:, 1:KI, :], fv(wx[:, 1:KI, :]),
                                         rrep)
                    st[ci].update(wx=wx)

                # ---- phase 3: gate matmuls + GRU tail ----
                for ci in range(NCHUNK):
                    cs = slice(ci * CB, (ci + 1) * CB)
                    h = hcur[ci]
                    wx = st[ci]["wx"]
                    ps_hn = pp.tile([128, 2, CB], f32, name=f"pshn_{t}_{ci}",
                                    tag="gps", bufs=3)
                    h8 = h8cur[ci]
                    for m in range(2):
                        for k in range(KH):
                            nc.tensor.matmul(
                                ps_hn[:, m, :], whhnt[:, k, ms(m)],
                                h[:, k, :], start=(k == 0), stop=(k == KH - 1))
                    ps_r = pp.tile([128, 2, CB], f32, name=f"psr_{t}_{ci}",
                                   tag="gps", bufs=3)
                    ps_z = pp.tile([128, 2, CB], f32, name=f"psz_{t}_{ci}",
                                   tag="gps", bufs=3)
                    # h-only whh matmuls of the m0 slices first (r and z are
                    # different banks, so both groups may be open at once):
                    # they keep the in-order PE stream fed while wx lands
                    for mm_t, base in ((ps_r, 0), (ps_z, 2)):
                        nc.tensor.matmul(
                            mm_t[:, 0, :], whh8t[:, 0:2, ms(base)],
                            h8[:, 0:2, :], start=True, stop=False,
                            perf_mode=DRM)
                    for mm_t, base in ((ps_r, 0), (ps_z, 2)):
                        for k in range(KI):
                            nc.tensor.matmul(
                                mm_t[:, 0, :], wiht[:, k, ms(base)],
                                wx[:, k, :], start=False, stop=(k == KI - 1))
                        nc.tensor.matmul(
                            mm_t[:, 1, :], whh8t[:, 0:2, ms(base + 1)],
                            h8[:, 0:2, :], start=True, stop=False,
                            perf_mode=DRM)
                        for k in range(KI):
                            nc.tensor.matmul(
                                mm_t[:, 1, :], wiht[:, k, ms(base + 1)],
                                wx[:, k, :], start=False, stop=(k == KI - 1))
                    ps_in = pp.tile([128, 2, CB], f32, name=f"psin_{t}_{ci}",
                                    tag="gps", bufs=3)
                    for m in range(2):
                        for k in range(KI):
                            nc.tensor.matmul(
                                ps_in[:, m, :], wiht[:, k, ms(4 + m)],
                                wx[:, k, :], start=(k == 0), stop=(k == KI - 1))

                    g = wp.tile([128, 4, CB], MMD, name=f"g_{t}_{ci}",
                                tag="g", bufs=3)
                    for m in range(4):
                        src_ps = ps_r if m < 2 else ps_z
                        nc.scalar.activation(g[:, m, :], src_ps[:, m % 2, :],
                                             AF.Tanh, bias=brzh[:, m:m + 1],
                                             scale=0.5)
                    t1h = wp.tile([128, 2, CB], MMD, name=f"t1h_{t}_{ci}",
                                  tag="t1h", bufs=3)
                    for m in range(2):
                        nc.vector.tensor_scalar(
                            out=t1h[:, m, :], in0=ps_hn[:, m, :],
                            scalar1=bhn[:, m:m + 1], scalar2=0.5,
                            op0=OP.add, op1=OP.mult)
                    # p = (i_n + b_in) + t1h is g-independent: compute it
                    # early so only two fp16 DVE ops trail the gate ACT
                    p_ = wp.tile([128, 2, CB], MMD, name=f"p_{t}_{ci}",
                                 tag="p", bufs=3)
                    for m in range(2):
                        nc.vector.scalar_tensor_tensor(
                            p_[:, m, :], ps_in[:, m, :], bin_[:, m:m + 1],
                            t1h[:, m, :], OP.add, OP.add)
                    t0h = wp.tile([128, 2, CB], MMD, name=f"t0h_{t}_{ci}",
                                  tag="t0h", bufs=3)
                    nc.vector.tensor_mul(t0h[:], t1h[:], g[:, 0:2, :])
                    s2 = wp.tile([128, 2, CB], MMD, name=f"s2_{t}_{ci}",
                                 tag="s2", bufs=3)
                    nc.vector.tensor_add(s2[:], t0h[:], p_[:])
                    n = wp.tile([128, 2, CB], MMD, name=f"n_{t}_{ci}",
                                tag="n", bufs=3)
                    nc.scalar.activation(n[:], s2[:], AF.Tanh)

                    zz = wp.tile([128, 2, CB], MMD, name=f"zz_{t}_{ci}",
                                 tag="zz", bufs=3)
                    nc.vector.tensor_scalar(
                        out=zz[:], in0=g[:, 2:4, :], scalar1=0.5, scalar2=0.5,
                        op0=OP.mult, op1=OP.add)
                    w1z = wp.tile([128, 2, CB], MMD, name=f"w1z_{t}_{ci}",
                                  tag="w1z", bufs=3)
                    nc.vector.tensor_scalar(
                        out=w1z[:], in0=g[:, 2:4, :], scalar1=-0.5,
                        scalar2=0.5, op0=OP.mult, op1=OP.add)
                    bzh = wp.tile([128, 2, CB], MMD, name=f"bzh_{t}_{ci}",
                                  tag="bzh", bufs=3)
                    nc.vector.tensor_mul(bzh[:], zz[:], fv(h[:]))
                    a4 = wp.tile([128, 2, CB], MMD, name=f"a4_{t}_{ci}",
                                 tag="a4", bufs=3)
                    nc.vector.tensor_mul(a4[:], w1z[:], n[:])
                    h_new = hp.tile([128, KH, CB], MMD, name=f"hn_{t}_{ci}",
                                    tag=f"h{ci}", bufs=2)
                    nc.vector.tensor_add(h_new[:], a4[:], bzh[:])
                    h8_new = hp.tile([128, KH, CB], F8D, name=f"h8n_{t}_{ci}",
                                     tag=f"h8{ci}", bufs=2)
                    nc.scalar.copy(h8_new[:], h_new[:])
                    hcur[ci] = h_new
                    h8cur[ci] = h8_new

                    nc.sync.dma_start(out=outT[t][:, :, cs], in_=h_new[:])

    nc.compile()
    return nc


# ---------------- host-side data prep ----------------

def _prep_core_inputs(x, h0, attn1_W, attn1_b, attn2_W, attn2_b, attn3_W,
                      attn3_b, W_ih, b_ih, W_hh, b_hh, t_steps=T):
    f4 = np.float32
    MMD, EVD = _dts()
    mnp = _np_dt(MMD)
    enp = _np_dt(EVD)
    x = np.asarray(x, f4)
    h0 = np.asarray(h0, f4)

    import ml_dtypes
    f8np = ml_dtypes.float8_e4m3
    A1 = np.asarray(attn1_W, f4)                       # [I, H]
    w1 = np.zeros((H, IP), f4)
    w1[:, :I] = A1.T                                   # lhsT[hh, ii]
    wat1 = np.ascontiguousarray(
        w1.reshape(KH, 128, IP).transpose(1, 0, 2)).astype(f8np)

    A2 = np.asarray(attn2_W, f4)                       # [I, I] (out, in)
    w2 = np.zeros((IP, IP), f4)
    w2[:I, :I] = A2.T                                  # lhsT[in, out]
    wat2 = np.ascontiguousarray(
        w2.reshape(KI, 128, IP).transpose(1, 0, 2)).astype(mnp)

    A3 = np.asarray(attn3_W, f4)
    w3 = np.zeros((IP, IP), f4)
    w3[:I, :I] = A3.T
    wat3 = np.ascontiguousarray(
        w3.reshape(KI, 128, IP).transpose(1, 0, 2)).astype(mnp)

    Wi = np.asarray(W_ih, f4)                          # [G, I]
    wi = np.zeros((IP, G), f4)
    wi[:I, :] = Wi.T
    wih = np.ascontiguousarray(
        wi.reshape(KI, 128, G).transpose(1, 0, 2)).astype(mnp)

    Wh = np.asarray(W_hh, f4)                          # [G, H]
    whT = Wh.T                                         # [H, G]
    whh8 = np.ascontiguousarray(
        whT[:, :2 * H].reshape(KH, 128, 2 * H).transpose(1, 0, 2)).astype(f8np)
    whhn = np.ascontiguousarray(
        whT[:, 2 * H:].reshape(KH, 128, H).transpose(1, 0, 2)).astype(mnp)

    onesw = np.ones((128, 128), enp)

    bu = np.zeros(IP, f4)
    bu[:I] = np.asarray(attn1_b, f4) + np.asarray(attn2_b, f4)
    bias_u = np.ascontiguousarray(bu.reshape(KI, 128).T)
    bvv = np.full(IP, -1e4, f4)
    bvv[:I] = np.asarray(attn3_b, f4) - 2.0   # shift-invariant, fp16 range
    bias_v = np.ascontiguousarray(bvv.reshape(KI, 128).T)
    brz = (np.asarray(b_ih, f4) + np.asarray(b_hh, f4))[:2 * H] * 0.5
    bias_rzh = np.ascontiguousarray(brz.reshape(4, 128).T)
    bias_hn = np.ascontiguousarray(
        np.asarray(b_hh, f4)[2 * H:].reshape(2, 128).T)
    bias_in = np.ascontiguousarray(
        np.asarray(b_ih, f4)[2 * H:].reshape(2, 128).T)

    h0r8 = h0.astype(f8np).reshape(NCORES, BS, KH, 128).transpose(0, 3, 2, 1)
    x16 = x[:, :t_steps, :].astype(mnp)
    xpad = np.pad(x16, ((0, 0), (0, 0), (0, IP - I)))
    # [NC, BS, T, KI, 128] -> [NC, T, 128, KI, BS]
    xr = xpad.reshape(NCORES, BS, t_steps, KI, 128).transpose(0, 2, 4, 3, 1)
    h0r = h0.astype(mnp).reshape(NCORES, BS, KH, 128).transpose(0, 3, 2, 1)

    shared = dict(wat1=wat1, wat2=wat2, wat3=wat3, wih=wih, whh8=whh8,
                  whhn=whhn, onesw=onesw, bias_u=bias_u, bias_v=bias_v,
                  bias_rzh=bias_rzh, bias_hn=bias_hn, bias_in=bias_in)
    in_maps = []
    for c in range(NCORES):
        m = dict(shared)
        m["xT"] = np.ascontiguousarray(xr[c])
        m["h0T"] = np.ascontiguousarray(h0r[c])
        m["h0T8"] = np.ascontiguousarray(h0r8[c])
        in_maps.append(m)
    return in_maps


def _gather(results, t_steps=T):
    outs = []
    for c in range(NCORES):
        o = np.asarray(results[c]["outT"], np.float32)
        outs.append(o.transpose(3, 0, 2, 1).reshape(BS, t_steps, H))
    return np.ascontiguousarray(np.concatenate(outs, axis=0))


def _get_nc(t_steps=T):
    key = ("nc", t_steps, DT)
    if key not in _STATE:
        _STATE[key] = _build(t_steps)
    return _STATE[key]


def run(inputs, trace=False, t_steps=T):
    from concourse.bass_utils import run_bass_kernel_spmd
    nc = _get_nc(t_steps)
    in_maps = _prep_core_inputs(t_steps=t_steps, **inputs)
    res = run_bass_kernel_spmd(nc, in_maps, list(range(NCORES)), trace=trace)
    return _gather(res.results, t_steps), res


def kernel(**inputs):
    out, _ = run(inputs, trace=False)
    return out



# revision 30
# speedup vs baseline: 1.0485x; 1.0337x over previous
"""Trainium2 Bass kernel for nn_Encoder_81595788689580.

Attention-gated GRU encoder: per time step
    w1 = h @ attn1_W.T + attn1_b
    w2 = x_t @ attn2_W.T + attn2_b
    v  = tanh(w1 + w2) @ attn3_W.T + attn3_b
    alpha = softmax(v, axis=feature)
    wx = x_t * alpha
    GRU cell (r, z, n) -> h_new
Output: [B, T, H] hidden states.

Strategy (8 NeuronCores, data-parallel over batch):
  - batch 4096 -> 512 rows per core; all weights replicated.
  - everything stored TRANSPOSED on chip: features on partitions, batch on
    the free dim. Every matmul is weights-stationary with batch as the
    moving dim, biases become per-partition ACT bias vectors, and no
    transposes are ever needed on device (host pre-/post-transposes).
  - feature dim I=320 zero-padded to 384 = 3x128 partition blocks; padded
    attn3_b rows are -1e4 so exp() of pad rows is exactly 0 and the
    softmax denominator is unaffected.
  - softmax over features is a partition reduction: an all-ones stationary
    matmul broadcasts the per-column denominator into all 128 partitions
    of one PSUM tile; max-subtraction is skipped (|v| <= ~8 in practice,
    exp stays finite, softmax is shift-invariant).
  - sigmoid is computed as 0.5*tanh(x/2)+0.5 so every ACT op uses the
    exp_and_others table set -- avoids ~2.7us ACT table swaps per step.
  - matmuls in fp16 (1 PE cycle/row, fast weight loads) with fp32 PSUM
    accumulation; attn3_b carries a -2 shift for fp16 exp range. DT="f32r" switches
    to float32r matmuls (~10x lower error, ~1.7x slower weight loads).
  - the 512-row batch runs as 2 independent chunks of 256 so the two
    recurrences pipeline against each other across engines.
"""

import numpy as np

B, T, I, H = 4096, 24, 320, 256
NCORES = 8
BS = B // NCORES          # 512 rows per core
IP = 384                  # I padded to 3*128
KI = IP // 128            # 3 feature blocks
KH = H // 128             # 2 hidden blocks
G = 3 * H                 # 768 gate rows
NCHUNK = 2
CB = BS // NCHUNK         # 256 batch columns per chunk

DT = "f16"                # "f16" | "f32r"

_STATE = {}


def _np_dt(mdt):
    from concourse import mybir
    return mybir.dt.np(mdt)


def _dts():
    from concourse import mybir
    if DT == "f16":
        return mybir.dt.float16, mybir.dt.float16
    return mybir.dt.float32r, mybir.dt.float32r


def _build(t_steps=T):
    import concourse.bass as bass
    import concourse.tile as tile
    from concourse import bacc, mybir

    f32 = mybir.dt.float32
    F8D = mybir.dt.float8e4
    DRM = mybir.MatmulPerfMode.DoubleRow
    MMD, EVD = _dts()
    AF = mybir.ActivationFunctionType
    OP = mybir.AluOpType

    nc = bacc.Bacc("TRN2", target_bir_lowering=False, debug=False,
                   num_devices=NCORES)

    xT = nc.dram_tensor("xT", [t_steps, 128, KI, BS], MMD,
                        kind="ExternalInput").ap()
    h0T = nc.dram_tensor("h0T", [128, KH, BS], MMD, kind="ExternalInput").ap()
    h0T8 = nc.dram_tensor("h0T8", [128, KH, BS], F8D, kind="ExternalInput").ap()
    wat1 = nc.dram_tensor("wat1", [128, KH, IP], F8D, kind="ExternalInput").ap()
    wat2 = nc.dram_tensor("wat2", [128, KI, IP], MMD, kind="ExternalInput").ap()
    wat3 = nc.dram_tensor("wat3", [128, KI, IP], MMD, kind="ExternalInput").ap()
    wih = nc.dram_tensor("wih", [128, KI, G], MMD, kind="ExternalInput").ap()
    whh8 = nc.dram_tensor("whh8", [128, KH, 2 * H], F8D,
                          kind="ExternalInput").ap()
    whhn = nc.dram_tensor("whhn", [128, KH, H], MMD,
                          kind="ExternalInput").ap()
    onesw = nc.dram_tensor("onesw", [128, 128], EVD, kind="ExternalInput").ap()
    bias_u_d = nc.dram_tensor("bias_u", [128, KI], f32, kind="ExternalInput").ap()
    bias_v_d = nc.dram_tensor("bias_v", [128, KI], f32, kind="ExternalInput").ap()
    # rz bias pre-halved for the tanh-based sigmoid
    bias_rzh_d = nc.dram_tensor("bias_rzh", [128, 4], f32,
                                kind="ExternalInput").ap()
    bias_hn_d = nc.dram_tensor("bias_hn", [128, 2], f32, kind="ExternalInput").ap()
    bias_in_d = nc.dram_tensor("bias_in", [128, 2], f32, kind="ExternalInput").ap()
    outT = nc.dram_tensor("outT", [t_steps, 128, KH, BS], MMD,
                          kind="ExternalOutput").ap()

    def fv(ap):
        # readable view for DVE of matmul-dtype tiles
        if DT == "f32r":
            return ap.bitcast(f32)
        return ap

    with tile.TileContext(nc) as tc:
        with tc.tile_pool(name="const", bufs=1) as cp, \
             tc.tile_pool(name="xs", bufs=1) as xp, \
             tc.tile_pool(name="hs", bufs=1) as hp, \
             tc.tile_pool(name="wk", bufs=1) as wp, \
             tc.tile_pool(name="ps", bufs=1, space="PSUM") as pp:

            w1t = cp.tile([128, KH, IP], F8D)
            w2t = cp.tile([128, KI, IP], MMD)
            w3t = cp.tile([128, KI, IP], MMD)
            wiht = cp.tile([128, KI, G], MMD)
            whh8t = cp.tile([128, KH, 2 * H], F8D)
            whhnt = cp.tile([128, KH, H], MMD)
            onest = cp.tile([128, 128], EVD)
            bu = cp.tile([128, KI], f32)
            bv = cp.tile([128, KI], f32)
            brzh = cp.tile([128, 4], f32)
            bhn = cp.tile([128, 2], f32)
            bin_ = cp.tile([128, 2], f32)
            # h0 + step-0 x first (they gate the first matmuls), then
            # weights ordered by first use, alternating the two HWDGE rings
            hcur, h8cur = [], []
            for ci in range(NCHUNK):
                hc = hp.tile([128, KH, CB], MMD, name=f"h_{ci}",
                             tag=f"h{ci}", bufs=2)
                nc.scalar.dma_start(
                    out=hc[:], in_=h0T[:, :, ci * CB:(ci + 1) * CB])
                hcur.append(hc)
                h8c = hp.tile([128, KH, CB], F8D, name=f"h8_{ci}",
                              tag=f"h8{ci}", bufs=2)
                nc.scalar.dma_start(
                    out=h8c[:], in_=h0T8[:, :, ci * CB:(ci + 1) * CB])
                h8cur.append(h8c)
            x_pre = xp.tile([128, KI, BS], MMD, name="x_pre", tag="x", bufs=4)
            nc.sync.dma_start(out=x_pre[:], in_=xT[0])
            for i, (dst, src) in enumerate([
                    (w2t, wat2), (w1t, wat1), (bu, bias_u_d),
                    (w3t, wat3), (bv, bias_v_d), (onest, onesw),
                    (whh8t, whh8), (whhnt, whhn), (wiht, wih),
                    (brzh, bias_rzh_d), (bhn, bias_hn_d),
                    (bin_, bias_in_d)]):
                eng = nc.sync if i % 2 == 0 else nc.scalar
                eng.dma_start(out=dst[:], in_=src)

            def ms(m):
                return slice(m * 128, (m + 1) * 128)

            for t in range(t_steps):
                if t == 0:
                    x_t = x_pre
                else:
                    x_t = xp.tile([128, KI, BS], MMD, name=f"x_{t}",
                                  tag="x", bufs=4)
                    nc.sync.dma_start(out=x_t[:], in_=xT[t])

                st = [{} for _ in range(NCHUNK)]

                # ---- phase 1: h-gate matmuls + attention stage 1 ----
                for ci in range(NCHUNK):
                    cs = slice(ci * CB, (ci + 1) * CB)
                    h = hcur[ci]
                    ps_u = [pp.tile([128, CB], f32,
                                    name=f"psu{m}_{t}_{ci}", tag="aps",
                                    bufs=5) for m in range(KI)]
                    h8 = h8cur[ci]
                    for m in range(KI):
                        for k in range(KI):
                            nc.tensor.matmul(
                                ps_u[m][:], w2t[:, k, ms(m)],
                                x_t[:, k, cs], start=(k == 0), stop=False)
                        nc.tensor.matmul(
                            ps_u[m][:], w1t[:, 0:2, ms(m)],
                            h8[:, 0:2, :], start=False, stop=True,
                            perf_mode=DRM)
                    u = wp.tile([128, KI, CB], MMD, name=f"u_{t}_{ci}",
                                tag="u", bufs=3)
                    for m in range(KI):
                        nc.scalar.activation(u[:, m, :], ps_u[m][:],
                                             AF.Tanh, bias=bu[:, m:m + 1])
                    st[ci].update(u=u)

                # ---- phase 2: v, softmax, wx ----
                for ci in range(NCHUNK):
                    cs = slice(ci * CB, (ci + 1) * CB)
                    u = st[ci]["u"]
                    ps_v = [pp.tile([128, CB], f32,
                                    name=f"psv{m}_{t}_{ci}", tag="aps",
                                    bufs=5) for m in range(KI)]
                    for m in range(KI):
                        for k in range(KI):
                            nc.tensor.matmul(
                                ps_v[m][:], w3t[:, k, ms(m)],
                                u[:, k, :], start=(k == 0), stop=(k == KI - 1))
                    ev = wp.tile([128, KI, CB], EVD, name=f"ev_{t}_{ci}",
                                 tag="ev", bufs=3)
                    for m in range(KI):
                        nc.scalar.activation(ev[:, m, :], ps_v[m][:],
                                             AF.Exp, bias=bv[:, m:m + 1])
                    ps_den = pp.tile([128, CB], f32, name=f"psden_{t}_{ci}",
                                     tag="aps", bufs=5)
                    for k in range(KI):
                        nc.tensor.matmul(ps_den[:], onest[:], ev[:, k, :],
                                         start=(k == 0), stop=(k == KI - 1))
                    rinv = wp.tile([128, CB], f32, name=f"rinv_{t}_{ci}",
                                   tag="rinv", bufs=3)
                    nc.vector.reciprocal_approx_fast(rinv[:], ps_den[:])
                    rinv16 = wp.tile([128, CB], MMD, name=f"rinv16_{t}_{ci}",
                                     tag="rinv16", bufs=3)
                    nc.vector.tensor_copy(rinv16[:], rinv[:])
                    wx = wp.tile([128, KI, CB], MMD, name=f"wx_{t}_{ci}",
                                 tag="wx", bufs=3)
                    nc.vector.tensor_mul(wx[:], fv(x_t[:, :, cs]), fv(ev[:]))
                    _r = rinv16[:]
                    nc.vector.tensor_mul(wx[:, 0, :], fv(wx[:, 0, :]), _r)
                    rrep = bass.AP(tensor=_r.tensor, offset=_r.offset,
                                   ap=[_r.ap[0], [0, KI - 1], _r.ap[1]])
                    nc.vector.tensor_mul(wx[:, 1:KI, :], fv(wx[:, 1:KI, :]),
                                         rrep)
                    st[ci].update(wx=wx)

                # ---- phase 3: gate matmuls + GRU tail ----
                for ci in range(NCHUNK):
                    cs = slice(ci * CB, (ci + 1) * CB)
                    h = hcur[ci]
                    wx = st[ci]["wx"]
                    ps_hn = pp.tile([128, 2, CB], f32, name=f"pshn_{t}_{ci}",
                                    tag="gps", bufs=3)
                    h8 = h8cur[ci]
                    for m in range(2):
                        for k in range(KH):
                            nc.tensor.matmul(
                                ps_hn[:, m, :], whhnt[:, k, ms(m)],
                                h[:, k, :], start=(k == 0), stop=(k == KH - 1))
                    ps_r = pp.tile([128, 2, CB], f32, name=f"psr_{t}_{ci}",
                                   tag="gps", bufs=3)
                    ps_z = pp.tile([128, 2, CB], f32, name=f"psz_{t}_{ci}",
                                   tag="gps", bufs=3)
                    # h-only whh matmuls of the m0 slices first (r and z are
                    # different banks, so both groups may be open at once):
                    # they keep the in-order PE stream fed while wx lands
                    for mm_t, base in ((ps_r, 0), (ps_z, 2)):
                        nc.tensor.matmul(
                            mm_t[:, 0, :], whh8t[:, 0:2, ms(base)],
                            h8[:, 0:2, :], start=True, stop=False,
                            perf_mode=DRM)
                    for mm_t, base in ((ps_r, 0), (ps_z, 2)):
                        for k in range(KI):
                            nc.tensor.matmul(
                                mm_t[:, 0, :], wiht[:, k, ms(base)],
                                wx[:, k, :], start=False, stop=(k == KI - 1))
                        nc.tensor.matmul(
                            mm_t[:, 1, :], whh8t[:, 0:2, ms(base + 1)],
                            h8[:, 0:2, :], start=True, stop=False,
                            perf_mode=DRM)
                        for k in range(KI):
                            nc.tensor.matmul(
                                mm_t[:, 1, :], wiht[:, k, ms(base + 1)],
                                wx[:, k, :], start=False, stop=(k == KI - 1))
                    ps_in = pp.tile([128, 2, CB], f32, name=f"psin_{t}_{ci}",
                                    tag="gps", bufs=3)
                    for m in range(2):
                        for k in range(KI):
                            nc.tensor.matmul(
                                ps_in[:, m, :], wiht[:, k, ms(4 + m)],
                                wx[:, k, :], start=(k == 0), stop=(k == KI - 1))

                    g = wp.tile([128, 4, CB], MMD, name=f"g_{t}_{ci}",
                                tag="g", bufs=3)
                    for m in range(4):
                        src_ps = ps_r if m < 2 else ps_z
                        nc.scalar.activation(g[:, m, :], src_ps[:, m % 2, :],
                                             AF.Tanh, bias=brzh[:, m:m + 1],
                                             scale=0.5)
                    t1h = wp.tile([128, 2, CB], MMD, name=f"t1h_{t}_{ci}",
                                  tag="t1h", bufs=3)
                    for m in range(2):
                        nc.vector.tensor_scalar(
                            out=t1h[:, m, :], in0=ps_hn[:, m, :],
                            scalar1=bhn[:, m:m + 1], scalar2=0.5,
                            op0=OP.add, op1=OP.mult)
                    # p = (i_n + b_in) + t1h is g-independent: compute it
                    # early so only two fp16 DVE ops trail the gate ACT
                    p_ = wp.tile([128, 2, CB], MMD, name=f"p_{t}_{ci}",
                                 tag="p", bufs=3)
                    for m in range(2):
                        nc.vector.scalar_tensor_tensor(
                            p_[:, m, :], ps_in[:, m, :], bin_[:, m:m + 1],
                            t1h[:, m, :], OP.add, OP.add)
                    t0h = wp.tile([128, 2, CB], MMD, name=f"t0h_{t}_{ci}",
                                  tag="t0h", bufs=3)
                    nc.vector.tensor_mul(t0h[:], t1h[:], g[:, 0:2, :])
                    s2 = wp.tile([128, 2, CB], MMD, name=f"s2_{t}_{ci}",
                                 tag="s2", bufs=3)
                    nc.vector.tensor_add(s2[:], t0h[:], p_[:])
                    n = wp.tile([128, 2, CB], MMD, name=f"n_{t}_{ci}",
                                tag="n", bufs=3)
                    nc.scalar.activation(n[:], s2[:], AF.Tanh)

                    zz = wp.tile([128, 2, CB], MMD, name=f"zz_{t}_{ci}",
                                 tag="zz", bufs=3)
                    nc.vector.tensor_scalar(
                        out=zz[:], in0=g[:, 2:4, :], scalar1=0.5, scalar2=0.5,
                        op0=OP.mult, op1=OP.add)
                    w1z = wp.tile([128, 2, CB], MMD, name=f"w1z_{t}_{ci}",
                                  tag="w1z", bufs=3)
                    nc.vector.tensor_scalar(
                        out=w1z[:], in0=g[:, 2:4, :], scalar1=-0.5,
                        scalar2=0.5, op0=OP.mult, op1=OP.add)
                    bzh = wp.tile([128, 2, CB], MMD, name=f"bzh_{t}_{ci}",
                                  tag="bzh", bufs=3)
                    nc.vector.tensor_mul(bzh[:], zz[:], fv(h[:]))
                    a4 = wp.tile([128, 2, CB], MMD, name=f"a4_{t}_{ci}",
                                 tag="a4", bufs=3)
                    nc.vector.tensor_mul(a4[:], w1z[:], n[:])
                    h_new = hp.tile([128, KH, CB], MMD, name=f"hn_{t}_{ci}",
                                    tag=f"h{ci}", bufs=2)
                    nc.vector.tensor_add(h_new[:], a4[:], bzh[:])
                    h8_new = hp.tile([128, KH, CB], F8D, name=f"h8n_{t}_{ci}",
                                     tag=f"h8{ci}", bufs=2)
                    nc.vector.tensor_add(h8_new[:], a4[:], bzh[:])
                    hcur[ci] = h_new
                    h8cur[ci] = h8_new

                    nc.sync.dma_start(out=outT[t][:, :, cs], in_=h_new[:])

    nc.compile()
    return nc


# ---------------- host-side data prep ----------------

def _prep_core_inputs(x, h0, attn1_W, attn1_b, attn2_W, attn2_b, attn3_W,
                      attn3_b, W_ih, b_ih, W_hh, b_hh, t_steps=T):
    f4 = np.float32
    MMD, EVD = _dts()
    mnp = _np_dt(MMD)
    enp = _np_dt(EVD)
    x = np.asarray(x, f4)
    h0 = np.asarray(h0, f4)

    import ml_dtypes
    f8np = ml_dtypes.float8_e4m3
    A1 = np.asarray(attn1_W, f4)                       # [I, H]
    w1 = np.zeros((H, IP), f4)
    w1[:, :I] = A1.T                                   # lhsT[hh, ii]
    wat1 = np.ascontiguousarray(
        w1.reshape(KH, 128, IP).transpose(1, 0, 2)).astype(f8np)

    A2 = np.asarray(attn2_W, f4)                       # [I, I] (out, in)
    w2 = np.zeros((IP, IP), f4)
    w2[:I, :I] = A2.T                                  # lhsT[in, out]
    wat2 = np.ascontiguousarray(
        w2.reshape(KI, 128, IP).transpose(1, 0, 2)).astype(mnp)

    A3 = np.asarray(attn3_W, f4)
    w3 = np.zeros((IP, IP), f4)
    w3[:I, :I] = A3.T
    wat3 = np.ascontiguousarray(
        w3.reshape(KI, 128, IP).transpose(1, 0, 2)).astype(mnp)

    Wi = np.asarray(W_ih, f4)                          # [G, I]
    wi = np.zeros((IP, G), f4)
    wi[:I, :] = Wi.T
    wih = np.ascontiguousarray(
        wi.reshape(KI, 128, G).transpose(1, 0, 2)).astype(mnp)

    Wh = np.asarray(W_hh, f4)                          # [G, H]
    whT = Wh.T                                         # [H, G]
    whh8 = np.ascontiguousarray(
        whT[:, :2 * H].reshape(KH, 128, 2 * H).transpose(1, 0, 2)).astype(f8np)
    whhn = np.ascontiguousarray(
        whT[:, 2 * H:].reshape(KH, 128, H).transpose(1, 0, 2)).astype(mnp)

    onesw = np.ones((128, 128), enp)

    bu = np.zeros(IP, f4)
    bu[:I] = np.asarray(attn1_b, f4) + np.asarray(attn2_b, f4)
    bias_u = np.ascontiguousarray(bu.reshape(KI, 128).T)
    bvv = np.full(IP, -1e4, f4)
    bvv[:I] = np.asarray(attn3_b, f4) - 2.0   # shift-invariant, fp16 range
    bias_v = np.ascontiguousarray(bvv.reshape(KI, 128).T)
    brz = (np.asarray(b_ih, f4) + np.asarray(b_hh, f4))[:2 * H] * 0.5
    bias_rzh = np.ascontiguousarray(brz.reshape(4, 128).T)
    bias_hn = np.ascontiguousarray(
        np.asarray(b_hh, f4)[2 * H:].reshape(2, 128).T)
    bias_in = np.ascontiguousarray(
        np.asarray(b_ih, f4)[2 * H:].reshape(2, 128).T)

    h0r8 = h0.astype(f8np).reshape(NCORES, BS, KH, 128).transpose(0, 3, 2, 1)
    x16 = x[:, :t_steps, :].astype(mnp)
    xpad = np.pad(x16, ((0, 0), (0, 0), (0, IP - I)))
    # [NC, BS, T, KI, 128] -> [NC, T, 128, KI, BS]
    xr = xpad.reshape(NCORES, BS, t_steps, KI, 128).transpose(0, 2, 4, 3, 1)
    h0r = h0.astype(mnp).reshape(NCORES, BS, KH, 128).transpose(0, 3, 2, 1)

    shared = dict(wat1=wat1, wat2=wat2, wat3=wat3, wih=wih, whh8=whh8,
                  whhn=whhn, onesw=onesw, bias_u=bias_u, bias_v=bias_v,
                  bias_rzh=bias_rzh, bias_hn=bias_hn, bias_in=bias_in)
    in_maps = []
    for c in range(NCORES):
        m = dict(shared)
        m["xT"] = np.ascontiguousarray(xr[c])
        m["h0T"] = np.ascontiguousarray(h0r[c])
        m["h0T8"] = np.ascontiguousarray(h0r8[c])
        in_maps.append(m)
    return in_maps


def _gather(results, t_steps=T):
    outs = []
    for c in range(NCORES):
        o = np.asarray(results[c]["outT"], np.float32)
        outs.append(o.transpose(3, 0, 2, 1).reshape(BS, t_steps, H))
    return np.ascontiguousarray(np.concatenate(outs, axis=0))


def _get_nc(t_steps=T):
    key = ("nc", t_steps, DT)
    if key not in _STATE:
        _STATE[key] = _build(t_steps)
    return _STATE[key]


def run(inputs, trace=False, t_steps=T):
    from concourse.bass_utils import run_bass_kernel_spmd
    nc = _get_nc(t_steps)
    in_maps = _prep_core_inputs(t_steps=t_steps, **inputs)
    res = run_bass_kernel_spmd(nc, in_maps, list(range(NCORES)), trace=trace)
    return _gather(res.results, t_steps), res


def kernel(**inputs):
    out, _ = run(inputs, trace=False)
    return out



# revision 31
# speedup vs baseline: 1.1145x; 1.0630x over previous
"""Trainium2 Bass kernel for nn_Encoder_81595788689580.

Attention-gated GRU encoder: per time step
    w1 = h @ attn1_W.T + attn1_b
    w2 = x_t @ attn2_W.T + attn2_b
    v  = tanh(w1 + w2) @ attn3_W.T + attn3_b
    alpha = softmax(v, axis=feature)
    wx = x_t * alpha
    GRU cell (r, z, n) -> h_new
Output: [B, T, H] hidden states.

Strategy (8 NeuronCores, data-parallel over batch):
  - batch 4096 -> 512 rows per core; all weights replicated.
  - everything stored TRANSPOSED on chip: features on partitions, batch on
    the free dim. Every matmul is weights-stationary with batch as the
    moving dim, biases become per-partition ACT bias vectors, and no
    transposes are ever needed on device (host pre-/post-transposes).
  - feature dim I=320 zero-padded to 384 = 3x128 partition blocks; padded
    attn3_b rows are -1e4 so exp() of pad rows is exactly 0 and the
    softmax denominator is unaffected.
  - softmax over features is a partition reduction: an all-ones stationary
    matmul broadcasts the per-column denominator into all 128 partitions
    of one PSUM tile; max-subtraction is skipped (|v| <= ~8 in practice,
    exp stays finite, softmax is shift-invariant).
  - sigmoid is computed as 0.5*tanh(x/2)+0.5 so every ACT op uses the
    exp_and_others table set -- avoids ~2.7us ACT table swaps per step.
  - matmuls in fp16 (1 PE cycle/row, fast weight loads) with fp32 PSUM
    accumulation; attn3_b carries a -2 shift for fp16 exp range. DT="f32r" switches
    to float32r matmuls (~10x lower error, ~1.7x slower weight loads).
  - the 512-row batch runs as 2 independent chunks of 256 so the two
    recurrences pipeline against each other across engines.
"""

import numpy as np

B, T, I, H = 4096, 24, 320, 256
NCORES = 8
BS = B // NCORES          # 512 rows per core
IP = 384                  # I padded to 3*128
KI = IP // 128            # 3 feature blocks
KH = H // 128             # 2 hidden blocks
G = 3 * H                 # 768 gate rows
NCHUNK = 2
CB = BS // NCHUNK         # 256 batch columns per chunk

DT = "f16"                # "f16" | "f32r"

_STATE = {}


def _np_dt(mdt):
    from concourse import mybir
    return mybir.dt.np(mdt)


def _dts():
    from concourse import mybir
    if DT == "f16":
        return mybir.dt.float16, mybir.dt.float16
    return mybir.dt.float32r, mybir.dt.float32r


def _build(t_steps=T):
    import concourse.bass as bass
    import concourse.tile as tile
    from concourse import bacc, mybir

    f32 = mybir.dt.float32
    MMD, EVD = _dts()
    AF = mybir.ActivationFunctionType
    OP = mybir.AluOpType

    nc = bacc.Bacc("TRN2", target_bir_lowering=False, debug=False,
                   num_devices=NCORES)

    xT = nc.dram_tensor("xT", [t_steps, 128, KI, BS], MMD,
                        kind="ExternalInput").ap()
    h0T = nc.dram_tensor("h0T", [128, KH, BS], MMD, kind="ExternalInput").ap()
    wat1 = nc.dram_tensor("wat1", [128, KH, IP], MMD, kind="ExternalInput").ap()
    wat2 = nc.dram_tensor("wat2", [128, KI, IP], MMD, kind="ExternalInput").ap()
    wat3 = nc.dram_tensor("wat3", [128, KI, IP], MMD, kind="ExternalInput").ap()
    wih = nc.dram_tensor("wih", [128, KI, G], MMD, kind="ExternalInput").ap()
    whh = nc.dram_tensor("whh", [128, KH, G], MMD, kind="ExternalInput").ap()
    onesw = nc.dram_tensor("onesw", [128, 128], EVD, kind="ExternalInput").ap()
    bias_u_d = nc.dram_tensor("bias_u", [128, KI], f32, kind="ExternalInput").ap()
    bias_v_d = nc.dram_tensor("bias_v", [128, KI], f32, kind="ExternalInput").ap()
    # rz bias pre-halved for the tanh-based sigmoid
    bias_rzh_d = nc.dram_tensor("bias_rzh", [128, 4], f32,
                                kind="ExternalInput").ap()
    bias_hn_d = nc.dram_tensor("bias_hn", [128, 2], f32, kind="ExternalInput").ap()
    bias_in_d = nc.dram_tensor("bias_in", [128, 2], f32, kind="ExternalInput").ap()
    outT = nc.dram_tensor("outT", [t_steps, 128, KH, BS], MMD,
                          kind="ExternalOutput").ap()

    def fv(ap):
        # readable view for DVE of matmul-dtype tiles
        if DT == "f32r":
            return ap.bitcast(f32)
        return ap

    with tile.TileContext(nc) as tc:
        with tc.tile_pool(name="const", bufs=1) as cp, \
             tc.tile_pool(name="xs", bufs=1) as xp, \
             tc.tile_pool(name="hs", bufs=1) as hp, \
             tc.tile_pool(name="wk", bufs=1) as wp, \
             tc.tile_pool(name="ps", bufs=1, space="PSUM") as pp:

            w1t = cp.tile([128, KH, IP], MMD)
            w2t = cp.tile([128, KI, IP], MMD)
            w3t = cp.tile([128, KI, IP], MMD)
            wiht = cp.tile([128, KI, G], MMD)
            whht = cp.tile([128, KH, G], MMD)
            onest = cp.tile([128, 128], EVD)
            bu = cp.tile([128, KI], f32)
            bv = cp.tile([128, KI], f32)
            brzh = cp.tile([128, 4], f32)
            bhn = cp.tile([128, 2], f32)
            bin_ = cp.tile([128, 2], f32)
            # h0 + step-0 x first (they gate the first matmuls), then
            # weights ordered by first use, alternating the two HWDGE rings
            hcur = []
            for ci in range(NCHUNK):
                hc = hp.tile([128, KH, CB], MMD, name=f"h_{ci}",
                             tag=f"h{ci}", bufs=2)
                nc.scalar.dma_start(
                    out=hc[:], in_=h0T[:, :, ci * CB:(ci + 1) * CB])
                hcur.append(hc)
            x_pre = xp.tile([128, KI, BS], MMD, name="x_pre", tag="x", bufs=4)
            nc.sync.dma_start(out=x_pre[:], in_=xT[0])
            for i, (dst, src) in enumerate([
                    (w2t, wat2), (w1t, wat1), (bu, bias_u_d),
                    (w3t, wat3), (bv, bias_v_d), (onest, onesw),
                    (whht, whh), (wiht, wih),
                    (brzh, bias_rzh_d), (bhn, bias_hn_d),
                    (bin_, bias_in_d)]):
                eng = nc.sync if i % 2 == 0 else nc.scalar
                eng.dma_start(out=dst[:], in_=src)

            def ms(m):
                return slice(m * 128, (m + 1) * 128)

            for t in range(t_steps):
                if t == 0:
                    x_t = x_pre
                else:
                    x_t = xp.tile([128, KI, BS], MMD, name=f"x_{t}",
                                  tag="x", bufs=4)
                    nc.sync.dma_start(out=x_t[:], in_=xT[t])

                st = [{} for _ in range(NCHUNK)]

                # ---- phase 1: h-gate matmuls + attention stage 1 ----
                for ci in range(NCHUNK):
                    cs = slice(ci * CB, (ci + 1) * CB)
                    h = hcur[ci]
                    ps_u = [pp.tile([128, CB], f32,
                                    name=f"psu{m}_{t}_{ci}", tag="aps",
                                    bufs=5) for m in range(KI)]
                    for m in range(KI):
                        for k in range(KI):
                            nc.tensor.matmul(
                                ps_u[m][:], w2t[:, k, ms(m)],
                                x_t[:, k, cs], start=(k == 0), stop=False)
                        for k in range(KH):
                            nc.tensor.matmul(
                                ps_u[m][:], w1t[:, k, ms(m)],
                                h[:, k, :], start=False, stop=(k == KH - 1))
                    u = wp.tile([128, KI, CB], MMD, name=f"u_{t}_{ci}",
                                tag="u", bufs=3)
                    for m in range(KI):
                        nc.scalar.activation(u[:, m, :], ps_u[m][:],
                                             AF.Tanh, bias=bu[:, m:m + 1])
                    st[ci].update(u=u)

                # ---- phase 2: v, softmax, wx ----
                for ci in range(NCHUNK):
                    cs = slice(ci * CB, (ci + 1) * CB)
                    u = st[ci]["u"]
                    ps_v = [pp.tile([128, CB], f32,
                                    name=f"psv{m}_{t}_{ci}", tag="aps",
                                    bufs=5) for m in range(KI)]
                    for m in range(KI):
                        for k in range(KI):
                            nc.tensor.matmul(
                                ps_v[m][:], w3t[:, k, ms(m)],
                                u[:, k, :], start=(k == 0), stop=(k == KI - 1))
                    ev = wp.tile([128, KI, CB], EVD, name=f"ev_{t}_{ci}",
                                 tag="ev", bufs=3)
                    for m in range(KI):
                        nc.scalar.activation(ev[:, m, :], ps_v[m][:],
                                             AF.Exp, bias=bv[:, m:m + 1])
                    ps_den = pp.tile([128, CB], f32, name=f"psden_{t}_{ci}",
                                     tag="aps", bufs=5)
                    for k in range(KI):
                        nc.tensor.matmul(ps_den[:], onest[:], ev[:, k, :],
                                         start=(k == 0), stop=(k == KI - 1))
                    rinv = wp.tile([128, CB], f32, name=f"rinv_{t}_{ci}",
                                   tag="rinv", bufs=3)
                    nc.vector.reciprocal_approx_fast(rinv[:], ps_den[:])
                    rinv16 = wp.tile([128, CB], MMD, name=f"rinv16_{t}_{ci}",
                                     tag="rinv16", bufs=3)
                    nc.vector.tensor_copy(rinv16[:], rinv[:])
                    wx = wp.tile([128, KI, CB], MMD, name=f"wx_{t}_{ci}",
                                 tag="wx", bufs=3)
                    nc.vector.tensor_mul(wx[:], fv(x_t[:, :, cs]), fv(ev[:]))
                    _r = rinv16[:]
                    nc.vector.tensor_mul(wx[:, 0, :], fv(wx[:, 0, :]), _r)
                    rrep = bass.AP(tensor=_r.tensor, offset=_r.offset,
                                   ap=[_r.ap[0], [0, KI - 1], _r.ap[1]])
                    nc.vector.tensor_mul(wx[:, 1:KI, :], fv(wx[:, 1:KI, :]),
                                         rrep)
                    st[ci].update(wx=wx)

                # ---- phase 3: gate matmuls + GRU tail ----
                for ci in range(NCHUNK):
                    cs = slice(ci * CB, (ci + 1) * CB)
                    h = hcur[ci]
                    wx = st[ci]["wx"]
                    ps_hn = pp.tile([128, 2, CB], f32, name=f"pshn_{t}_{ci}",
                                    tag="gps", bufs=3)
                    for m in range(2):
                        for k in range(KH):
                            nc.tensor.matmul(
                                ps_hn[:, m, :], whht[:, k, ms(4 + m)],
                                h[:, k, :], start=(k == 0), stop=(k == KH - 1))
                    ps_r = pp.tile([128, 2, CB], f32, name=f"psr_{t}_{ci}",
                                   tag="gps", bufs=3)
                    ps_z = pp.tile([128, 2, CB], f32, name=f"psz_{t}_{ci}",
                                   tag="gps", bufs=3)
                    # h-only whh matmuls of the m0 slices first (r and z are
                    # different banks, so both groups may be open at once):
                    # they keep the in-order PE stream fed while wx lands
                    for mm_t, base in ((ps_r, 0), (ps_z, 2)):
                        for k in range(KH):
                            nc.tensor.matmul(
                                mm_t[:, 0, :], whht[:, k, ms(base)],
                                h[:, k, :], start=(k == 0), stop=False)
                    for mm_t, base in ((ps_r, 0), (ps_z, 2)):
                        for k in range(KI):
                            nc.tensor.matmul(
                                mm_t[:, 0, :], wiht[:, k, ms(base)],
                                wx[:, k, :], start=False, stop=(k == KI - 1))
                        for k in range(KH):
                            nc.tensor.matmul(
                                mm_t[:, 1, :], whht[:, k, ms(base + 1)],
                                h[:, k, :], start=(k == 0), stop=False)
                        for k in range(KI):
                            nc.tensor.matmul(
                                mm_t[:, 1, :], wiht[:, k, ms(base + 1)],
                                wx[:, k, :], start=False, stop=(k == KI - 1))
                    ps_in = pp.tile([128, 2, CB], f32, name=f"psin_{t}_{ci}",
                                    tag="gps", bufs=3)
                    for m in range(2):
                        for k in range(KI):
                            nc.tensor.matmul(
                                ps_in[:, m, :], wiht[:, k, ms(4 + m)],
                                wx[:, k, :], start=(k == 0), stop=(k == KI - 1))

                    g = wp.tile([128, 4, CB], MMD, name=f"g_{t}_{ci}",
                                tag="g", bufs=3)
                    for m in range(4):
                        src_ps = ps_r if m < 2 else ps_z
                        nc.scalar.activation(g[:, m, :], src_ps[:, m % 2, :],
                                             AF.Tanh, bias=brzh[:, m:m + 1],
                                             scale=0.5)
                    t1h = wp.tile([128, 2, CB], MMD, name=f"t1h_{t}_{ci}",
                                  tag="t1h", bufs=3)
                    for m in range(2):
                        nc.vector.tensor_scalar(
                            out=t1h[:, m, :], in0=ps_hn[:, m, :],
                            scalar1=bhn[:, m:m + 1], scalar2=0.5,
                            op0=OP.add, op1=OP.mult)
                    # p = (i_n + b_in) + t1h is g-independent: compute it
                    # early so only two fp16 DVE ops trail the gate ACT
                    p_ = wp.tile([128, 2, CB], MMD, name=f"p_{t}_{ci}",
                                 tag="p", bufs=3)
                    for m in range(2):
                        nc.vector.scalar_tensor_tensor(
                            p_[:, m, :], ps_in[:, m, :], bin_[:, m:m + 1],
                            t1h[:, m, :], OP.add, OP.add)
                    t0h = wp.tile([128, 2, CB], MMD, name=f"t0h_{t}_{ci}",
                                  tag="t0h", bufs=3)
                    nc.vector.tensor_mul(t0h[:], t1h[:], g[:, 0:2, :])
                    s2 = wp.tile([128, 2, CB], MMD, name=f"s2_{t}_{ci}",
                                 tag="s2", bufs=3)
                    nc.vector.tensor_add(s2[:], t0h[:], p_[:])
                    n = wp.tile([128, 2, CB], MMD, name=f"n_{t}_{ci}",
                                tag="n", bufs=3)
                    nc.scalar.activation(n[:], s2[:], AF.Tanh)

                    zz = wp.tile([128, 2, CB], MMD, name=f"zz_{t}_{ci}",
                                 tag="zz", bufs=3)
                    nc.vector.tensor_scalar(
                        out=zz[:], in0=g[:, 2:4, :], scalar1=0.5, scalar2=0.5,
                        op0=OP.mult, op1=OP.add)
                    w1z = wp.tile([128, 2, CB], MMD, name=f"w1z_{t}_{ci}",
                                  tag="w1z", bufs=3)
                    nc.vector.tensor_scalar(
                        out=w1z[:], in0=g[:, 2:4, :], scalar1=-0.5,
                        scalar2=0.5, op0=OP.mult, op1=OP.add)
                    bzh = wp.tile([128, 2, CB], MMD, name=f"bzh_{t}_{ci}",
                                  tag="bzh", bufs=3)
                    nc.vector.tensor_mul(bzh[:], zz[:], fv(h[:]))
                    a4 = wp.tile([128, 2, CB], MMD, name=f"a4_{t}_{ci}",
                                 tag="a4", bufs=3)
                    nc.vector.tensor_mul(a4[:], w1z[:], n[:])
                    h_new = hp.tile([128, KH, CB], MMD, name=f"hn_{t}_{ci}",
                                    tag=f"h{ci}", bufs=2)
                    nc.vector.tensor_add(h_new[:], a4[:], bzh[:])
                    hcur[ci] = h_new

                    nc.sync.dma_start(out=outT[t][:, :, cs], in_=h_new[:])

    nc.compile()
    return nc


# ---------------- host-side data prep ----------------

def _prep_core_inputs(x, h0, attn1_W, attn1_b, attn2_W, attn2_b, attn3_W,
                      attn3_b, W_ih, b_ih, W_hh, b_hh, t_steps=T):
    f4 = np.float32
    MMD, EVD = _dts()
    mnp = _np_dt(MMD)
    enp = _np_dt(EVD)
    x = np.asarray(x, f4)
    h0 = np.asarray(h0, f4)

    A1 = np.asarray(attn1_W, f4)                       # [I, H]
    w1 = np.zeros((H, IP), f4)
    w1[:, :I] = A1.T                                   # lhsT[hh, ii]
    wat1 = np.ascontiguousarray(
        w1.reshape(KH, 128, IP).transpose(1, 0, 2)).astype(mnp)

    A2 = np.asarray(attn2_W, f4)                       # [I, I] (out, in)
    w2 = np.zeros((IP, IP), f4)
    w2[:I, :I] = A2.T                                  # lhsT[in, out]
    wat2 = np.ascontiguousarray(
        w2.reshape(KI, 128, IP).transpose(1, 0, 2)).astype(mnp)

    A3 = np.asarray(attn3_W, f4)
    w3 = np.zeros((IP, IP), f4)
    w3[:I, :I] = A3.T
    wat3 = np.ascontiguousarray(
        w3.reshape(KI, 128, IP).transpose(1, 0, 2)).astype(mnp)

    Wi = np.asarray(W_ih, f4)                          # [G, I]
    wi = np.zeros((IP, G), f4)
    wi[:I, :] = Wi.T
    wih = np.ascontiguousarray(
        wi.reshape(KI, 128, G).transpose(1, 0, 2)).astype(mnp)

    Wh = np.asarray(W_hh, f4)                          # [G, H]
    whh = np.ascontiguousarray(
        Wh.T.reshape(KH, 128, G).transpose(1, 0, 2)).astype(mnp)

    onesw = np.ones((128, 128), enp)

    bu = np.zeros(IP, f4)
    bu[:I] = np.asarray(attn1_b, f4) + np.asarray(attn2_b, f4)
    bias_u = np.ascontiguousarray(bu.reshape(KI, 128).T)
    bvv = np.full(IP, -1e4, f4)
    bvv[:I] = np.asarray(attn3_b, f4) - 2.0   # shift-invariant, fp16 range
    bias_v = np.ascontiguousarray(bvv.reshape(KI, 128).T)
    brz = (np.asarray(b_ih, f4) + np.asarray(b_hh, f4))[:2 * H] * 0.5
    bias_rzh = np.ascontiguousarray(brz.reshape(4, 128).T)
    bias_hn = np.ascontiguousarray(
        np.asarray(b_hh, f4)[2 * H:].reshape(2, 128).T)
    bias_in = np.ascontiguousarray(
        np.asarray(b_ih, f4)[2 * H:].reshape(2, 128).T)

    x16 = x[:, :t_steps, :].astype(mnp)
    xpad = np.pad(x16, ((0, 0), (0, 0), (0, IP - I)))
    # [NC, BS, T, KI, 128] -> [NC, T, 128, KI, BS]
    xr = xpad.reshape(NCORES, BS, t_steps, KI, 128).transpose(0, 2, 4, 3, 1)
    h0r = h0.astype(mnp).reshape(NCORES, BS, KH, 128).transpose(0, 3, 2, 1)

    shared = dict(wat1=wat1, wat2=wat2, wat3=wat3, wih=wih, whh=whh,
                  onesw=onesw, bias_u=bias_u, bias_v=bias_v,
                  bias_rzh=bias_rzh, bias_hn=bias_hn, bias_in=bias_in)
    in_maps = []
    for c in range(NCORES):
        m = dict(shared)
        m["xT"] = np.ascontiguousarray(xr[c])
        m["h0T"] = np.ascontiguousarray(h0r[c])
        in_maps.append(m)
    return in_maps


def _gather(results, t_steps=T):
    outs = []
    for c in range(NCORES):
        o = np.asarray(results[c]["outT"], np.float32)
        outs.append(o.transpose(3, 0, 2, 1).reshape(BS, t_steps, H))
    return np.ascontiguousarray(np.concatenate(outs, axis=0))


def _get_nc(t_steps=T):
    key = ("nc", t_steps, DT)
    if key not in _STATE:
        _STATE[key] = _build(t_steps)
    return _STATE[key]


def run(inputs, trace=False, t_steps=T):
    from concourse.bass_utils import run_bass_kernel_spmd
    nc = _get_nc(t_steps)
    in_maps = _prep_core_inputs(t_steps=t_steps, **inputs)
    res = run_bass_kernel_spmd(nc, in_maps, list(range(NCORES)), trace=trace)
    return _gather(res.results, t_steps), res


def kernel(**inputs):
    out, _ = run(inputs, trace=False)
    return out

